# revision 5
# baseline (speedup 1.0000x reference)
import sys

sys.path.insert(0, "/opt/trn_rl_repo")
import numpy as np
import ml_dtypes

from concourse import bacc, tile, mybir
from concourse.bass_utils import run_bass_kernel_spmd

BF16 = ml_dtypes.bfloat16
N_CORES = 8
N, K, C_IN, H, W = 512, 4, 3, 21, 21
HID, C2, ACT_DIM = 64, 16, 5
OBS_R = (H // 2, W // 2)
PIX = H * W          # 441
PIXP = 448           # padded pixel count (multiple of 8)
NR = PIXP // 8       # 56 rounds of 8 pixels
A_PC = N // N_CORES  # 64 agents per core
I_PC = A_PC * K      # 256 images per core

_CACHE = {}
LAST_RESULT = None


def _ensure_ntff_hook():
    """This image's antenv lacks axon_hooks; inject a shim so
    run_bass_kernel_spmd's trace path works (exec_time_ns)."""
    import types

    try:
        import antenv

        if hasattr(antenv, "axon_hooks"):
            return
        from trn_agent_boot.trn_boot import _ntff_profile_via_ctypes

        mod = types.ModuleType("antenv.axon_hooks")
        _h = [_ntff_profile_via_ctypes("/opt/axon/libaxon_pjrt.so")]
        mod.set_axon_ntff_profile_hook = lambda h: _h.__setitem__(0, h)
        mod.get_axon_ntff_profile_hook = lambda: _h[0]
        sys.modules["antenv.axon_hooks"] = mod
        antenv.axon_hooks = mod
    except Exception:
        pass


def _build_nc():
    f32 = mybir.dt.float32
    bf16 = mybir.dt.bfloat16
    RELU = mybir.ActivationFunctionType.Relu
    ADD = mybir.AluOpType.add
    MAX = mybir.AluOpType.max

    nc = bacc.Bacc("TRN2", target_bir_lowering=False, debug=False, num_devices=N_CORES)
    xim = nc.declare_dram_parameter("xim", [NR, 128, 2 * I_PC], bf16, isOutput=False)
    w1s = nc.declare_dram_parameter("w1s", [128, 64], bf16, isOutput=False)
    w2b = nc.declare_dram_parameter("w2b", [128, 32], bf16, isOutput=False)
    wc = nc.declare_dram_parameter("wc", [128, NR * ACT_DIM], bf16, isOutput=False)
    b1 = nc.declare_dram_parameter("b1", [128, 1], f32, isOutput=False)
    b2 = nc.declare_dram_parameter("b2", [128, 1], f32, isOutput=False)
    mT = nc.declare_dram_parameter("mT", [4, 128, A_PC], f32, isOutput=False)
    hb = nc.declare_dram_parameter("hb", [A_PC, ACT_DIM], f32, isOutput=False)
    ey = nc.declare_dram_parameter("ey", [ACT_DIM, ACT_DIM], f32, isOutput=False)
    out = nc.declare_dram_parameter("out", [A_PC, ACT_DIM], f32, isOutput=True)

    with tile.TileContext(nc) as tc:
        with (
            tc.tile_pool(name="w", bufs=1) as wp,
            tc.tile_pool(name="x", bufs=2) as xp,
            tc.tile_pool(name="r", bufs=6) as rp,
            tc.tile_pool(name="pp", bufs=2) as pp,
            tc.tile_pool(name="sm", bufs=1) as sm,
            tc.tile_pool(name="ps1", bufs=4, space="PSUM") as ps1,
            tc.tile_pool(name="ps2", bufs=2, space="PSUM") as ps2,
            tc.tile_pool(name="psp", bufs=1, space="PSUM") as psp,
            tc.tile_pool(name="psq", bufs=1, space="PSUM") as psq,
            tc.tile_pool(name="dram", bufs=1, space="DRAM") as dp,
        ):
            w1t = wp.tile([128, 64], bf16)
            nc.sync.dma_start(w1t[:], w1s[:])
            w2t = wp.tile([128, 32], bf16)
            nc.sync.dma_start(w2t[:], w2b[:])
            wct = wp.tile([128, NR * ACT_DIM], bf16)
            nc.sync.dma_start(wct[:], wc[:])
            b1t = wp.tile([128, 1], f32)
            nc.sync.dma_start(b1t[:], b1[:])
            b2t = wp.tile([128, 1], f32)
            nc.sync.dma_start(b2t[:], b2[:])
            mt = wp.tile([128, 4, A_PC], f32)
            nc.sync.dma_start(mt[:], mT[:].rearrange("k j i -> j k i"))
            hbt = wp.tile([A_PC, ACT_DIM], f32)
            nc.sync.dma_start(hbt[:], hb[:])
            eyt = wp.tile([ACT_DIM, ACT_DIM], f32)
            nc.sync.dma_start(eyt[:], ey[:])

            # projection accumulator, lives for the whole image loop
            qacc = psp.tile([ACT_DIM, I_PC], f32, tag="qacc")

            SG = 8  # rounds per DMA super-group (1 MiB per dma_start)
            for g in range(NR // SG):
                xt = xp.tile([128, SG, 2 * I_PC], bf16, tag="xt")
                nc.sync.dma_start(
                    xt[:], xim[g * SG : (g + 1) * SG, :, :].rearrange("r p c -> p r c")
                )
                for rr in range(SG):
                    R = g * SG + rr
                    # conv1: 4 row-strip matmuls; strips 0,1 -> bank A, strips 2,3 -> bank B
                    psA = ps1.tile([128, 2 * I_PC], f32, tag="c1")
                    psB = ps1.tile([128, 2 * I_PC], f32, tag="c1")
                    for s in range(4):
                        bank = psA if s < 2 else psB
                        e = s % 2
                        nc.tensor.matmul(
                            bank[64 * e : 64 * (e + 1), :],
                            w1t[32 * s : 32 * s + 27, :],
                            xt[32 * s : 32 * s + 27, rr, :],
                            start=True,
                            stop=True,
                            tile_position=(32 * s, 64 * e),
                        )
                    # relu1 + bias, psum -> sbuf bf16; split across ACT and DVE
                    rA = rp.tile([128, 2 * I_PC], bf16, tag="r")
                    rB = rp.tile([128, 2 * I_PC], bf16, tag="r")
                    nc.scalar.activation(rA[:], psA[:], RELU, bias=b1t[:, 0:1])
                    nc.vector.tensor_scalar(rB[:], psB[:], b1t[:, 0:1], 0.0, ADD, MAX)
                    # conv2: 4 col-tiled block-diag matmuls -> pixels 8R..8R+8 packed
                    if rr % 2 == 0:
                        psC = ps2.tile([128, 2, I_PC], f32, tag="c2")
                    for j in range(4):
                        src = rA if j % 2 == 0 else rB
                        hh = j // 2
                        nc.tensor.matmul(
                            psC[32 * j : 32 * (j + 1), rr % 2, :],
                            w2t[:, :],
                            src[:, hh * I_PC : (hh + 1) * I_PC],
                            start=True,
                            stop=True,
                            tile_position=(0, 32 * j),
                        )
                    if rr % 2 == 1:
                        # relu2 + bias for two rounds at once; alternate engines
                        pt = pp.tile([128, 2, I_PC], bf16, tag="pt")
                        if (R // 2) % 2 == 0:
                            nc.scalar.activation(pt[:], psC[:], RELU, bias=b2t[:, 0:1])
                        else:
                            nc.vector.tensor_scalar(
                                pt[:], psC[:], b2t[:, 0:1], 0.0, ADD, MAX
                            )
                        # projection: accumulate q[5, I_PC] over all 56 chunks
                        for t in range(2):
                            Rp = R - 1 + t
                            nc.tensor.matmul(
                                qacc[:],
                                wct[:, Rp * ACT_DIM : (Rp + 1) * ACT_DIM],
                                pt[:, t, :],
                                start=(Rp == 0),
                                stop=(Rp == NR - 1),
                            )

            # mean over K folded as sum (wc pre-scaled by 1/K): q5[5, 64]
            q5 = sm.tile([ACT_DIM, A_PC], f32, tag="q5")
            nc.vector.tensor_reduce(
                q5[:],
                qacc[:].rearrange("p (a k) -> p a k", k=K),
                mybir.AxisListType.X,
                ADD,
            )
            # transpose to [64, 5] via PE (q5.T @ I)
            psT = psq.tile([A_PC, ACT_DIM], f32, tag="q")
            nc.tensor.matmul(psT[:], q5[:], eyt[:], start=True, stop=True)
            qsb = sm.tile([A_PC, ACT_DIM], f32, tag="qsb")
            nc.vector.tensor_copy(qsb[:], psT[:])

            agi = dp.tile([A_PC, ACT_DIM], f32)
            ago = dp.tile([N, ACT_DIM], f32)
            nc.gpsimd.dma_start(agi[:], qsb[:])
            nc.gpsimd.collective_compute(
                "AllGather",
                mybir.AluOpType.bypass,
                replica_groups=[list(range(N_CORES))],
                ins=[agi[:].opt()],
                outs=[ago[:].opt()],
            )
            qall = sm.tile([128, 4, ACT_DIM], f32, tag="qall")
            nc.gpsimd.dma_start(qall[:], ago[:].rearrange("(k j) a -> j k a", j=128))

            # masked aggregation: Q[i, a] = sum_j mask[i, j] q[j, a]
            psM = psq.tile([A_PC, ACT_DIM], f32, tag="q")
            for kc in range(4):
                nc.tensor.matmul(
                    psM[:],
                    mt[:, kc, :],
                    qall[:, kc, :],
                    start=(kc == 0),
                    stop=(kc == 3),
                )
            osb = sm.tile([A_PC, ACT_DIM], f32, tag="osb")
            nc.vector.tensor_tensor(osb[:], psM[:], hbt[:], ADD)
            nc.sync.dma_start(out[:], osb[:])

    nc.compile()
    return nc


def _host_prep(obs, action, state, conv1_w, conv1_b, conv2_w, conv2_b,
               obs_w, obs_b, act_w, act_b, val_w, val_b, adv_w, adv_b):
    f = np.float32
    obs = np.asarray(obs, f)
    action = np.asarray(action).astype(np.int64)
    state = np.asarray(state).astype(np.int64)
    conv1_w = np.asarray(conv1_w, f)
    conv1_b = np.asarray(conv1_b, f)
    conv2_w = np.asarray(conv2_w, f)
    conv2_b = np.asarray(conv2_b, f)
    obs_w = np.asarray(obs_w, f)
    obs_b = np.asarray(obs_b, f)
    act_w = np.asarray(act_w, f)
    act_b = np.asarray(act_b, f)
    val_w = np.asarray(val_w, f)
    val_b = np.asarray(val_b, f)
    adv_w = np.asarray(adv_w, f)
    adv_b = np.asarray(adv_b, f)

    # dueling head folded into a single linear: Q = latent @ Wq.T + bq
    Wq = val_w[0][None, :] + adv_w - adv_w.mean(axis=0)[None, :]  # [5, 32]
    bq = val_b[0] + adv_b - adv_b.mean()                          # [5]
    Wqo, Wqa = Wq[:, :16], Wq[:, 16:]
    W_combo = (Wqo @ obs_w) / K                                   # [5, 7056]

    aoh = np.zeros((N, ACT_DIM), f)
    aoh[np.arange(N), action] = 1.0
    a_enc = aoh @ act_w.T + act_b                                 # [512, 16]
    h = obs_b @ Wqo.T + a_enc @ Wqa.T                             # [512, 5]

    d = np.abs(state[:, None, :] - state[None, :, :])
    within = (d[..., 0] <= OBS_R[0]) & (d[..., 1] <= OBS_R[1])
    upper = np.triu(np.ones((N, N), bool), 1)
    mask = (np.eye(N, dtype=bool) | (within & upper)).astype(f)   # [512, 512]
    hbias = mask @ h + bq[None, :]                                # [512, 5]

    # device weight layouts
    w1 = conv1_w.reshape(HID, C_IN * 9)                           # [64, 27]
    w1s = np.zeros((128, 64), f)
    for s in range(4):
        w1s[32 * s : 32 * s + 27] = w1.T
    w2 = conv2_w.reshape(C2, HID)                                 # [16, 64]
    w2b = np.zeros((128, 32), f)
    w2b[0:64, 0:16] = w2.T
    w2b[64:128, 16:32] = w2.T
    Wc3 = W_combo.reshape(ACT_DIM, C2, PIX)                       # [5, 16, 441]
    wcf = np.zeros((128, NR, ACT_DIM), f)
    for G in range(NR):
        for q in range(8):
            p = 8 * G + q
            if p < PIX:
                wcf[16 * q : 16 * (q + 1), G, :] = Wc3[:, :, p].T
    wc = wcf.reshape(128, NR * ACT_DIM)
    b1t = np.tile(conv1_b, 2).reshape(128, 1).astype(f)
    b2t = np.tile(conv2_b, 8).reshape(128, 1).astype(f)

    # im2col: K27[(c,dh,dw), pix, img]
    from numpy.lib.stride_tricks import sliding_window_view

    obs_im = obs.reshape(N * K, C_IN, H, W)
    obs_p = np.pad(obs_im, ((0, 0), (0, 0), (1, 1), (1, 1)))
    win = sliding_window_view(obs_p, (3, 3), axis=(2, 3))         # [NK, 3, 21, 21, 3, 3]
    K27 = win.transpose(1, 4, 5, 2, 3, 0).reshape(27, PIX, N * K)
    K27p = np.zeros((27, PIXP, N * K), f)
    K27p[:, :PIX] = K27
    Kv = K27p.reshape(27, NR, 2, 4, N * K)                        # (k, R, h, s, img)

    eye5 = np.eye(ACT_DIM, dtype=f)

    in_maps = []
    for r in range(N_CORES):
        i0, i1 = r * I_PC, (r + 1) * I_PC
        ximr = np.zeros((NR, 128, 2 * I_PC), BF16)
        for s in range(4):
            blk = Kv[:, :, :, s, i0:i1].transpose(1, 0, 2, 3).reshape(NR, 27, 2 * I_PC)
            ximr[:, 32 * s : 32 * s + 27, :] = blk.astype(BF16)
        a0 = r * A_PC
        mTr = np.ascontiguousarray(
            mask[a0 : a0 + A_PC, :].T.reshape(4, 128, A_PC)
        ).astype(f)
        in_maps.append(
            {
                "xim": ximr,
                "w1s": w1s.astype(BF16),
                "w2b": w2b.astype(BF16),
                "wc": wc.astype(BF16),
                "b1": b1t,
                "b2": b2t,
                "mT": mTr,
                "hb": np.ascontiguousarray(hbias[a0 : a0 + A_PC]).astype(f),
                "ey": eye5,
            }
        )
    return in_maps


def kernel(**inputs):
    global LAST_RESULT
    _ensure_ntff_hook()
    in_maps = _host_prep(**inputs)
    if "nc" not in _CACHE:
        _CACHE["nc"] = _build_nc()
    nc = _CACHE["nc"]
    res = run_bass_kernel_spmd(nc, in_maps, core_ids=list(range(N_CORES)))
    LAST_RESULT = res
    outp = np.concatenate([res.results[r]["out"] for r in range(N_CORES)], axis=0)
    return outp.astype(np.float32)


# revision 11
# speedup vs baseline: 1.3352x; 1.3352x over previous
import sys

sys.path.insert(0, "/opt/trn_rl_repo")
import numpy as np
import ml_dtypes

from concourse import bacc, tile, mybir
from concourse.bass_utils import run_bass_kernel_spmd

BF16 = ml_dtypes.bfloat16
N_CORES = 8
N, K, C_IN, H, W = 512, 4, 3, 21, 21
HID, C2, ACT_DIM = 64, 16, 5
OBS_R = (H // 2, W // 2)
PIX = H * W          # 441
PIXP = 448           # padded pixel count (multiple of 8)
NR = PIXP // 8       # 56 rounds of 8 pixels
A_PC = N // N_CORES  # 64 agents per core
I_PC = A_PC * K      # 256 images per core

_CACHE = {}
LAST_RESULT = None

import os as _os

F_WARM = _os.environ.get("K_WARM", "1") == "1"
F_PIPE = _os.environ.get("K_PIPE", "1") == "1"
F_DMAEND = _os.environ.get("K_DMAEND", "1") == "1"


def _ensure_ntff_hook():
    """This image's antenv lacks axon_hooks; inject a shim so
    run_bass_kernel_spmd's trace path works (exec_time_ns)."""
    import types

    try:
        import antenv

        if hasattr(antenv, "axon_hooks"):
            return
        from trn_agent_boot.trn_boot import _ntff_profile_via_ctypes

        mod = types.ModuleType("antenv.axon_hooks")
        _h = [_ntff_profile_via_ctypes("/opt/axon/libaxon_pjrt.so")]
        mod.set_axon_ntff_profile_hook = lambda h: _h.__setitem__(0, h)
        mod.get_axon_ntff_profile_hook = lambda: _h[0]
        sys.modules["antenv.axon_hooks"] = mod
        antenv.axon_hooks = mod
    except Exception:
        pass


def _build_nc():
    f32 = mybir.dt.float32
    bf16 = mybir.dt.bfloat16
    RELU = mybir.ActivationFunctionType.Relu
    ADD = mybir.AluOpType.add
    MAX = mybir.AluOpType.max

    nc = bacc.Bacc("TRN2", target_bir_lowering=False, debug=False, num_devices=N_CORES)
    xim = nc.declare_dram_parameter("xim", [NR, 128, 2 * I_PC], bf16, isOutput=False)
    w1s = nc.declare_dram_parameter("w1s", [128, 64], bf16, isOutput=False)
    w2b = nc.declare_dram_parameter("w2b", [128, 32], bf16, isOutput=False)
    wc = nc.declare_dram_parameter("wc", [128, NR * ACT_DIM], bf16, isOutput=False)
    b1 = nc.declare_dram_parameter("b1", [128, 1], f32, isOutput=False)
    b2 = nc.declare_dram_parameter("b2", [128, 1], f32, isOutput=False)
    mT = nc.declare_dram_parameter("mT", [4, 128, A_PC], f32, isOutput=False)
    hb = nc.declare_dram_parameter("hb", [A_PC, ACT_DIM], f32, isOutput=False)
    ey = nc.declare_dram_parameter("ey", [ACT_DIM, ACT_DIM], f32, isOutput=False)
    out = nc.declare_dram_parameter("out", [A_PC, ACT_DIM], f32, isOutput=True)

    with tile.TileContext(nc) as tc:
        with (
            tc.tile_pool(name="w", bufs=1) as wp,
            tc.tile_pool(name="x", bufs=2) as xp,
            tc.tile_pool(name="r", bufs=8) as rp,
            tc.tile_pool(name="pp", bufs=3) as pp,
            tc.tile_pool(name="sm", bufs=1) as sm,
            tc.tile_pool(name="ps1", bufs=4, space="PSUM") as ps1,
            tc.tile_pool(name="ps2", bufs=2, space="PSUM") as ps2,
            tc.tile_pool(name="psp", bufs=1, space="PSUM") as psp,
            tc.tile_pool(name="psq", bufs=1, space="PSUM") as psq,
            tc.tile_pool(name="dram", bufs=1, space="DRAM") as dp,
        ):
            w1t = wp.tile([128, 64], bf16)
            nc.sync.dma_start(w1t[:], w1s[:])
            w2t = wp.tile([128, 32], bf16)
            nc.sync.dma_start(w2t[:], w2b[:])
            wct = wp.tile([128, NR * ACT_DIM], bf16)
            nc.sync.dma_start(wct[:], wc[:])
            b1t = wp.tile([128, 1], f32)
            nc.sync.dma_start(b1t[:], b1[:])
            b2t = wp.tile([128, 1], f32)
            nc.sync.dma_start(b2t[:], b2[:])

            # projection accumulator, lives for the whole image loop
            qacc = psp.tile([ACT_DIM, I_PC], f32, tag="qacc")

            # PE warm-up: dummy matmuls while the first input DMA is in flight
            # (HAM unthrottles 1.2 -> 2.4 GHz after ~3.4us of sustained work)
            if F_WARM:
                warm = psq.tile([64, NR * ACT_DIM], f32, tag="q")
                for _ in range(24):
                    nc.tensor.matmul(
                        warm[:], w1t[:, :], wct[:, :], start=True, stop=True
                    )

            SG = 4  # rounds per DMA super-group (512 KiB per dma_start)
            D = 1 if F_PIPE else 0   # conv2 delay (software pipeline depth)
            prev = {}         # R -> (rA, rB)
            pend = None       # (pair, pt) projection deferred one pair
            psC = None
            for R in range(NR + D):
                if R < NR:
                    if R % SG == 0:
                        g = R // SG
                        xt = xp.tile([128, SG, 2 * I_PC], bf16, tag="xt")
                        nc.sync.dma_start(
                            xt[:],
                            xim[g * SG : (g + 1) * SG, :, :].rearrange(
                                "r p c -> p r c"
                            ),
                        )
                    rr = R % SG
                    # conv1: 4 row-strip matmuls; strips 0,1 -> bank A, 2,3 -> bank B
                    psA = ps1.tile([128, 2 * I_PC], f32, tag="c1")
                    psB = ps1.tile([128, 2 * I_PC], f32, tag="c1")
                    for s in range(4):
                        bank = psA if s < 2 else psB
                        e = s % 2
                        nc.tensor.matmul(
                            bank[64 * e : 64 * (e + 1), :],
                            w1t[32 * s : 32 * s + 27, :],
                            xt[32 * s : 32 * s + 27, rr, :],
                            start=True,
                            stop=True,
                            tile_position=(32 * s, 64 * e),
                        )
                    # relu1 + bias, psum -> sbuf bf16; split across ACT and DVE
                    rA = rp.tile([128, 2 * I_PC], bf16, tag="r")
                    rB = rp.tile([128, 2 * I_PC], bf16, tag="r")
                    nc.scalar.activation(rA[:], psA[:], RELU, bias=b1t[:, 0:1])
                    nc.vector.tensor_scalar(rB[:], psB[:], b1t[:, 0:1], 0.0, ADD, MAX)
                    prev[R] = (rA, rB)
                # conv2 for round R-D (pipelined: PE starts conv1(R) while
                # relu1(R-D) drains on ACT/DVE)
                Rp = R - D
                if 0 <= Rp < NR:
                    rAp, rBp = prev.pop(Rp)
                    if Rp % 2 == 0:
                        psC = ps2.tile([128, 2, I_PC], f32, tag="c2")
                    for j in range(4):
                        src = rAp if j % 2 == 0 else rBp
                        hh = j // 2
                        nc.tensor.matmul(
                            psC[32 * j : 32 * (j + 1), Rp % 2, :],
                            w2t[:, :],
                            src[:, hh * I_PC : (hh + 1) * I_PC],
                            start=True,
                            stop=True,
                            tile_position=(0, 32 * j),
                        )
                    if Rp % 2 == 1:
                        # relu2 + bias for two rounds at once; alternate engines
                        pt = pp.tile([128, 2, I_PC], bf16, tag="pt")
                        if (Rp // 2) % 2 == 0:
                            nc.scalar.activation(pt[:], psC[:], RELU, bias=b2t[:, 0:1])
                        else:
                            nc.vector.tensor_scalar(
                                pt[:], psC[:], b2t[:, 0:1], 0.0, ADD, MAX
                            )
                        if not F_PIPE:
                            P = Rp // 2
                            for t in range(2):
                                Rq = 2 * P + t
                                nc.tensor.matmul(
                                    qacc[:],
                                    wct[:, Rq * ACT_DIM : (Rq + 1) * ACT_DIM],
                                    pt[:, t, :],
                                    start=(Rq == 0),
                                    stop=(Rq == NR - 1),
                                )
                        else:
                            if pend is not None:
                                P, ptp = pend
                                for t in range(2):
                                    Rq = 2 * P + t
                                    nc.tensor.matmul(
                                        qacc[:],
                                        wct[:, Rq * ACT_DIM : (Rq + 1) * ACT_DIM],
                                        ptp[:, t, :],
                                        start=(Rq == 0),
                                        stop=False,
                                    )
                            pend = (Rp // 2, pt)
            if F_PIPE:
                # flush the last pending projection pair
                P, ptp = pend
                for t in range(2):
                    Rq = 2 * P + t
                    nc.tensor.matmul(
                        qacc[:],
                        wct[:, Rq * ACT_DIM : (Rq + 1) * ACT_DIM],
                        ptp[:, t, :],
                        start=False,
                        stop=(Rq == NR - 1),
                    )
            if not F_DMAEND:
                pass  # (placeholder; loads below happen here regardless)
            mt = wp.tile([128, 4, A_PC], f32)
            nc.sync.dma_start(mt[:], mT[:].rearrange("k j i -> j k i"))
            hbt = wp.tile([A_PC, ACT_DIM], f32)
            nc.sync.dma_start(hbt[:], hb[:])
            eyt = wp.tile([ACT_DIM, ACT_DIM], f32)
            nc.sync.dma_start(eyt[:], ey[:])
            # mean over K folded as sum (wc pre-scaled by 1/K): q5[5, 64]
            q5 = sm.tile([ACT_DIM, A_PC], f32, tag="q5")
            nc.vector.tensor_reduce(
                q5[:],
                qacc[:].rearrange("p (a k) -> p a k", k=K),
                mybir.AxisListType.X,
                ADD,
            )
            # transpose to [64, 5] via PE (q5.T @ I)
            psT = psq.tile([A_PC, ACT_DIM], f32, tag="q")
            nc.tensor.matmul(psT[:], q5[:], eyt[:], start=True, stop=True)
            qsb = sm.tile([A_PC, ACT_DIM], f32, tag="qsb")
            nc.vector.tensor_copy(qsb[:], psT[:])

            agi = dp.tile([A_PC, ACT_DIM], f32)
            ago = dp.tile([N, ACT_DIM], f32)
            nc.gpsimd.dma_start(agi[:], qsb[:])
            nc.gpsimd.collective_compute(
                "AllGather",
                mybir.AluOpType.bypass,
                replica_groups=[list(range(N_CORES))],
                ins=[agi[:].opt()],
                outs=[ago[:].opt()],
            )
            qall = sm.tile([128, 4, ACT_DIM], f32, tag="qall")
            nc.gpsimd.dma_start(qall[:], ago[:].rearrange("(k j) a -> j k a", j=128))

            # masked aggregation: Q[i, a] = sum_j mask[i, j] q[j, a]
            psM = psq.tile([A_PC, ACT_DIM], f32, tag="q")
            for kc in range(4):
                nc.tensor.matmul(
                    psM[:],
                    mt[:, kc, :],
                    qall[:, kc, :],
                    start=(kc == 0),
                    stop=(kc == 3),
                )
            osb = sm.tile([A_PC, ACT_DIM], f32, tag="osb")
            nc.vector.tensor_tensor(osb[:], psM[:], hbt[:], ADD)
            nc.sync.dma_start(out[:], osb[:])

    nc.compile()
    return nc


def _host_prep(obs, action, state, conv1_w, conv1_b, conv2_w, conv2_b,
               obs_w, obs_b, act_w, act_b, val_w, val_b, adv_w, adv_b):
    f = np.float32
    obs = np.asarray(obs, f)
    action = np.asarray(action).astype(np.int64)
    state = np.asarray(state).astype(np.int64)
    conv1_w = np.asarray(conv1_w, f)
    conv1_b = np.asarray(conv1_b, f)
    conv2_w = np.asarray(conv2_w, f)
    conv2_b = np.asarray(conv2_b, f)
    obs_w = np.asarray(obs_w, f)
    obs_b = np.asarray(obs_b, f)
    act_w = np.asarray(act_w, f)
    act_b = np.asarray(act_b, f)
    val_w = np.asarray(val_w, f)
    val_b = np.asarray(val_b, f)
    adv_w = np.asarray(adv_w, f)
    adv_b = np.asarray(adv_b, f)

    # dueling head folded into a single linear: Q = latent @ Wq.T + bq
    Wq = val_w[0][None, :] + adv_w - adv_w.mean(axis=0)[None, :]  # [5, 32]
    bq = val_b[0] + adv_b - adv_b.mean()                          # [5]
    Wqo, Wqa = Wq[:, :16], Wq[:, 16:]
    W_combo = (Wqo @ obs_w) / K                                   # [5, 7056]

    aoh = np.zeros((N, ACT_DIM), f)
    aoh[np.arange(N), action] = 1.0
    a_enc = aoh @ act_w.T + act_b                                 # [512, 16]
    h = obs_b @ Wqo.T + a_enc @ Wqa.T                             # [512, 5]

    d = np.abs(state[:, None, :] - state[None, :, :])
    within = (d[..., 0] <= OBS_R[0]) & (d[..., 1] <= OBS_R[1])
    upper = np.triu(np.ones((N, N), bool), 1)
    mask = (np.eye(N, dtype=bool) | (within & upper)).astype(f)   # [512, 512]
    hbias = mask @ h + bq[None, :]                                # [512, 5]

    # device weight layouts
    w1 = conv1_w.reshape(HID, C_IN * 9)                           # [64, 27]
    w1s = np.zeros((128, 64), f)
    for s in range(4):
        w1s[32 * s : 32 * s + 27] = w1.T
    w2 = conv2_w.reshape(C2, HID)                                 # [16, 64]
    w2b = np.zeros((128, 32), f)
    w2b[0:64, 0:16] = w2.T
    w2b[64:128, 16:32] = w2.T
    Wc3 = W_combo.reshape(ACT_DIM, C2, PIX)                       # [5, 16, 441]
    wcf = np.zeros((128, NR, ACT_DIM), f)
    for G in range(NR):
        for q in range(8):
            p = 8 * G + q
            if p < PIX:
                wcf[16 * q : 16 * (q + 1), G, :] = Wc3[:, :, p].T
    wc = wcf.reshape(128, NR * ACT_DIM)
    b1t = np.tile(conv1_b, 2).reshape(128, 1).astype(f)
    b2t = np.tile(conv2_b, 8).reshape(128, 1).astype(f)

    # im2col: K27[(c,dh,dw), pix, img]
    from numpy.lib.stride_tricks import sliding_window_view

    obs_im = obs.reshape(N * K, C_IN, H, W)
    obs_p = np.pad(obs_im, ((0, 0), (0, 0), (1, 1), (1, 1)))
    win = sliding_window_view(obs_p, (3, 3), axis=(2, 3))         # [NK, 3, 21, 21, 3, 3]
    K27 = win.transpose(1, 4, 5, 2, 3, 0).reshape(27, PIX, N * K)
    K27p = np.zeros((27, PIXP, N * K), f)
    K27p[:, :PIX] = K27
    Kv = K27p.reshape(27, NR, 2, 4, N * K)                        # (k, R, h, s, img)

    eye5 = np.eye(ACT_DIM, dtype=f)

    in_maps = []
    for r in range(N_CORES):
        i0, i1 = r * I_PC, (r + 1) * I_PC
        ximr = np.zeros((NR, 128, 2 * I_PC), BF16)
        for s in range(4):
            blk = Kv[:, :, :, s, i0:i1].transpose(1, 0, 2, 3).reshape(NR, 27, 2 * I_PC)
            ximr[:, 32 * s : 32 * s + 27, :] = blk.astype(BF16)
        a0 = r * A_PC
        mTr = np.ascontiguousarray(
            mask[a0 : a0 + A_PC, :].T.reshape(4, 128, A_PC)
        ).astype(f)
        in_maps.append(
            {
                "xim": ximr,
                "w1s": w1s.astype(BF16),
                "w2b": w2b.astype(BF16),
                "wc": wc.astype(BF16),
                "b1": b1t,
                "b2": b2t,
                "mT": mTr,
                "hb": np.ascontiguousarray(hbias[a0 : a0 + A_PC]).astype(f),
                "ey": eye5,
            }
        )
    return in_maps


def kernel(**inputs):
    global LAST_RESULT
    _ensure_ntff_hook()
    in_maps = _host_prep(**inputs)
    import time as _time
    print("build start", _time.strftime("%H:%M:%S"), flush=True)
    if "nc" not in _CACHE:
        _CACHE["nc"] = _build_nc()
    nc = _CACHE["nc"]
    print("build done", _time.strftime("%H:%M:%S"), flush=True)
    import time as _time
    print("run start", _time.strftime("%H:%M:%S"), flush=True)
    res = run_bass_kernel_spmd(nc, in_maps, core_ids=list(range(N_CORES)))
    print("run done", _time.strftime("%H:%M:%S"), flush=True)
    LAST_RESULT = res
    outp = np.concatenate([res.results[r]["out"] for r in range(N_CORES)], axis=0)
    return outp.astype(np.float32)


# revision 13
# speedup vs baseline: 1.3723x; 1.0278x over previous
import sys

sys.path.insert(0, "/opt/trn_rl_repo")
import numpy as np
import ml_dtypes

from concourse import bacc, tile, mybir
from concourse.bass_utils import run_bass_kernel_spmd

BF16 = ml_dtypes.bfloat16
N_CORES = 8
N, K, C_IN, H, W = 512, 4, 3, 21, 21
HID, C2, ACT_DIM = 64, 16, 5
OBS_R = (H // 2, W // 2)
PIX = H * W          # 441
PIXP = 448           # padded pixel count (multiple of 8)
NR = PIXP // 8       # 56 rounds of 8 pixels
A_PC = N // N_CORES  # 64 agents per core
I_PC = A_PC * K      # 256 images per core

_CACHE = {}
LAST_RESULT = None

import os as _os

F_WARM = _os.environ.get("K_WARM", "0") == "1"
F_PIPE = _os.environ.get("K_PIPE", "1") == "1"
F_DMAEND = _os.environ.get("K_DMAEND", "1") == "1"


def _ensure_ntff_hook():
    """This image's antenv lacks axon_hooks; inject a shim so
    run_bass_kernel_spmd's trace path works (exec_time_ns)."""
    import types

    try:
        import antenv

        if hasattr(antenv, "axon_hooks"):
            return
        from trn_agent_boot.trn_boot import _ntff_profile_via_ctypes

        mod = types.ModuleType("antenv.axon_hooks")
        _h = [_ntff_profile_via_ctypes("/opt/axon/libaxon_pjrt.so")]
        mod.set_axon_ntff_profile_hook = lambda h: _h.__setitem__(0, h)
        mod.get_axon_ntff_profile_hook = lambda: _h[0]
        sys.modules["antenv.axon_hooks"] = mod
        antenv.axon_hooks = mod
    except Exception:
        pass


def _build_nc():
    f32 = mybir.dt.float32
    bf16 = mybir.dt.bfloat16
    RELU = mybir.ActivationFunctionType.Relu
    ADD = mybir.AluOpType.add
    MAX = mybir.AluOpType.max

    nc = bacc.Bacc("TRN2", target_bir_lowering=False, debug=False, num_devices=N_CORES)
    xim = nc.declare_dram_parameter("xim", [NR, 128, 2 * I_PC], bf16, isOutput=False)
    w1s = nc.declare_dram_parameter("w1s", [128, 64], bf16, isOutput=False)
    w2b = nc.declare_dram_parameter("w2b", [128, 32], bf16, isOutput=False)
    wc = nc.declare_dram_parameter("wc", [128, NR * ACT_DIM], bf16, isOutput=False)
    b1 = nc.declare_dram_parameter("b1", [128, 1], f32, isOutput=False)
    b2 = nc.declare_dram_parameter("b2", [128, 1], f32, isOutput=False)
    mT = nc.declare_dram_parameter("mT", [4, 128, A_PC], f32, isOutput=False)
    hb = nc.declare_dram_parameter("hb", [A_PC, ACT_DIM], f32, isOutput=False)
    ey = nc.declare_dram_parameter("ey", [ACT_DIM, ACT_DIM], f32, isOutput=False)
    out = nc.declare_dram_parameter("out", [A_PC, ACT_DIM], f32, isOutput=True)

    with tile.TileContext(nc) as tc:
        with (
            tc.tile_pool(name="w", bufs=1) as wp,
            tc.tile_pool(name="x", bufs=2) as xp,
            tc.tile_pool(name="r", bufs=8) as rp,
            tc.tile_pool(name="pp", bufs=3) as pp,
            tc.tile_pool(name="sm", bufs=1) as sm,
            tc.tile_pool(name="ps1", bufs=4, space="PSUM") as ps1,
            tc.tile_pool(name="ps2", bufs=2, space="PSUM") as ps2,
            tc.tile_pool(name="psp", bufs=1, space="PSUM") as psp,
            tc.tile_pool(name="psq", bufs=1, space="PSUM") as psq,
            tc.tile_pool(name="dram", bufs=1, space="DRAM") as dp,
        ):
            w1t = wp.tile([128, 64], bf16)
            nc.sync.dma_start(w1t[:], w1s[:])
            w2t = wp.tile([128, 32], bf16)
            nc.sync.dma_start(w2t[:], w2b[:])
            wct = wp.tile([128, NR * ACT_DIM], bf16)
            nc.sync.dma_start(wct[:], wc[:])
            b1t = wp.tile([128, 1], f32)
            nc.sync.dma_start(b1t[:], b1[:])
            b2t = wp.tile([128, 1], f32)
            nc.sync.dma_start(b2t[:], b2[:])

            # projection accumulator, lives for the whole image loop
            qacc = psp.tile([ACT_DIM, I_PC], f32, tag="qacc")

            # PE warm-up: dummy matmuls while the first input DMA is in flight
            # (HAM unthrottles 1.2 -> 2.4 GHz after ~3.4us of sustained work)
            if F_WARM:
                warm = psq.tile([64, NR * ACT_DIM], f32, tag="q")
                for _ in range(24):
                    nc.tensor.matmul(
                        warm[:], w1t[:, :], wct[:, :], start=True, stop=True
                    )

            SG = 4  # rounds per DMA super-group (512 KiB per dma_start)
            D = 1 if F_PIPE else 0   # conv2 delay (software pipeline depth)
            prev = {}         # R -> (rA, rB)
            pend = None       # (pair, pt) projection deferred one pair
            psC = None
            for R in range(NR + D):
                if R < NR:
                    if R % SG == 0:
                        g = R // SG
                        xt = xp.tile([128, SG, 2 * I_PC], bf16, tag="xt")
                        if g == 0:
                            h = SG // 2
                            nc.sync.dma_start(
                                xt[:, 0:h, :],
                                xim[0:h, :, :].rearrange("r p c -> p r c"),
                            )
                            nc.sync.dma_start(
                                xt[:, h:SG, :],
                                xim[h:SG, :, :].rearrange("r p c -> p r c"),
                            )
                        else:
                            nc.sync.dma_start(
                                xt[:],
                                xim[g * SG : (g + 1) * SG, :, :].rearrange(
                                    "r p c -> p r c"
                                ),
                            )
                    rr = R % SG
                    # conv1: 4 row-strip matmuls; strips 0,1 -> bank A, 2,3 -> bank B
                    psA = ps1.tile([128, 2 * I_PC], f32, tag="c1")
                    psB = ps1.tile([128, 2 * I_PC], f32, tag="c1")
                    for s in range(4):
                        bank = psA if s < 2 else psB
                        e = s % 2
                        nc.tensor.matmul(
                            bank[64 * e : 64 * (e + 1), :],
                            w1t[32 * s : 32 * s + 27, :],
                            xt[32 * s : 32 * s + 27, rr, :],
                            start=True,
                            stop=True,
                            tile_position=(32 * s, 64 * e),
                        )
                    # relu1 + bias, psum -> sbuf bf16; split across ACT and DVE
                    rA = rp.tile([128, 2 * I_PC], bf16, tag="r")
                    rB = rp.tile([128, 2 * I_PC], bf16, tag="r")
                    nc.scalar.activation(rA[:], psA[:], RELU, bias=b1t[:, 0:1])
                    nc.vector.tensor_scalar(rB[:], psB[:], b1t[:, 0:1], 0.0, ADD, MAX)
                    prev[R] = (rA, rB)
                # conv2 for round R-D (pipelined: PE starts conv1(R) while
                # relu1(R-D) drains on ACT/DVE)
                Rp = R - D
                if 0 <= Rp < NR:
                    rAp, rBp = prev.pop(Rp)
                    if Rp % 2 == 0:
                        psC = ps2.tile([128, 2, I_PC], f32, tag="c2")
                    for j in range(4):
                        src = rAp if j % 2 == 0 else rBp
                        hh = j // 2
                        nc.tensor.matmul(
                            psC[32 * j : 32 * (j + 1), Rp % 2, :],
                            w2t[:, :],
                            src[:, hh * I_PC : (hh + 1) * I_PC],
                            start=True,
                            stop=True,
                            tile_position=(0, 32 * j),
                        )
                    if Rp % 2 == 1:
                        # relu2 + bias for two rounds at once; alternate engines
                        pt = pp.tile([128, 2, I_PC], bf16, tag="pt")
                        nc.scalar.activation(pt[:], psC[:], RELU, bias=b2t[:, 0:1])
                        if not F_PIPE:
                            P = Rp // 2
                            for t in range(2):
                                Rq = 2 * P + t
                                nc.tensor.matmul(
                                    qacc[:],
                                    wct[:, Rq * ACT_DIM : (Rq + 1) * ACT_DIM],
                                    pt[:, t, :],
                                    start=(Rq == 0),
                                    stop=(Rq == NR - 1),
                                )
                        else:
                            if pend is not None:
                                P, ptp = pend
                                for t in range(2):
                                    Rq = 2 * P + t
                                    nc.tensor.matmul(
                                        qacc[:],
                                        wct[:, Rq * ACT_DIM : (Rq + 1) * ACT_DIM],
                                        ptp[:, t, :],
                                        start=(Rq == 0),
                                        stop=False,
                                    )
                            pend = (Rp // 2, pt)
            if F_PIPE:
                # flush the last pending projection pair
                P, ptp = pend
                for t in range(2):
                    Rq = 2 * P + t
                    nc.tensor.matmul(
                        qacc[:],
                        wct[:, Rq * ACT_DIM : (Rq + 1) * ACT_DIM],
                        ptp[:, t, :],
                        start=False,
                        stop=(Rq == NR - 1),
                    )
            if not F_DMAEND:
                pass  # (placeholder; loads below happen here regardless)
            mt = wp.tile([128, 4, A_PC], f32)
            nc.sync.dma_start(mt[:], mT[:].rearrange("k j i -> j k i"))
            hbt = wp.tile([A_PC, ACT_DIM], f32)
            nc.sync.dma_start(hbt[:], hb[:])
            eyt = wp.tile([ACT_DIM, ACT_DIM], f32)
            nc.sync.dma_start(eyt[:], ey[:])
            # mean over K folded as sum (wc pre-scaled by 1/K): q5[5, 64]
            q5 = sm.tile([ACT_DIM, A_PC], f32, tag="q5")
            nc.vector.tensor_reduce(
                q5[:],
                qacc[:].rearrange("p (a k) -> p a k", k=K),
                mybir.AxisListType.X,
                ADD,
            )
            # transpose to [64, 5] via PE (q5.T @ I)
            psT = psq.tile([A_PC, ACT_DIM], f32, tag="q")
            nc.tensor.matmul(psT[:], q5[:], eyt[:], start=True, stop=True)
            qsb = sm.tile([A_PC, ACT_DIM], f32, tag="qsb")
            nc.vector.tensor_copy(qsb[:], psT[:])

            agi = dp.tile([A_PC, ACT_DIM], f32)
            ago = dp.tile([N, ACT_DIM], f32)
            nc.gpsimd.dma_start(agi[:], qsb[:])
            nc.gpsimd.collective_compute(
                "AllGather",
                mybir.AluOpType.bypass,
                replica_groups=[list(range(N_CORES))],
                ins=[agi[:].opt()],
                outs=[ago[:].opt()],
            )
            qall = sm.tile([128, 4, ACT_DIM], f32, tag="qall")
            nc.gpsimd.dma_start(qall[:], ago[:].rearrange("(k j) a -> j k a", j=128))

            # masked aggregation: Q[i, a] = sum_j mask[i, j] q[j, a]
            psM = psq.tile([A_PC, ACT_DIM], f32, tag="q")
            for kc in range(4):
                nc.tensor.matmul(
                    psM[:],
                    mt[:, kc, :],
                    qall[:, kc, :],
                    start=(kc == 0),
                    stop=(kc == 3),
                )
            osb = sm.tile([A_PC, ACT_DIM], f32, tag="osb")
            nc.vector.tensor_tensor(osb[:], psM[:], hbt[:], ADD)
            nc.sync.dma_start(out[:], osb[:])

    nc.compile()
    return nc


def _host_prep(obs, action, state, conv1_w, conv1_b, conv2_w, conv2_b,
               obs_w, obs_b, act_w, act_b, val_w, val_b, adv_w, adv_b):
    f = np.float32
    obs = np.asarray(obs, f)
    action = np.asarray(action).astype(np.int64)
    state = np.asarray(state).astype(np.int64)
    conv1_w = np.asarray(conv1_w, f)
    conv1_b = np.asarray(conv1_b, f)
    conv2_w = np.asarray(conv2_w, f)
    conv2_b = np.asarray(conv2_b, f)
    obs_w = np.asarray(obs_w, f)
    obs_b = np.asarray(obs_b, f)
    act_w = np.asarray(act_w, f)
    act_b = np.asarray(act_b, f)
    val_w = np.asarray(val_w, f)
    val_b = np.asarray(val_b, f)
    adv_w = np.asarray(adv_w, f)
    adv_b = np.asarray(adv_b, f)

    # dueling head folded into a single linear: Q = latent @ Wq.T + bq
    Wq = val_w[0][None, :] + adv_w - adv_w.mean(axis=0)[None, :]  # [5, 32]
    bq = val_b[0] + adv_b - adv_b.mean()                          # [5]
    Wqo, Wqa = Wq[:, :16], Wq[:, 16:]
    W_combo = (Wqo @ obs_w) / K                                   # [5, 7056]

    aoh = np.zeros((N, ACT_DIM), f)
    aoh[np.arange(N), action] = 1.0
    a_enc = aoh @ act_w.T + act_b                                 # [512, 16]
    h = obs_b @ Wqo.T + a_enc @ Wqa.T                             # [512, 5]

    d = np.abs(state[:, None, :] - state[None, :, :])
    within = (d[..., 0] <= OBS_R[0]) & (d[..., 1] <= OBS_R[1])
    upper = np.triu(np.ones((N, N), bool), 1)
    mask = (np.eye(N, dtype=bool) | (within & upper)).astype(f)   # [512, 512]
    hbias = mask @ h + bq[None, :]                                # [512, 5]

    # device weight layouts
    w1 = conv1_w.reshape(HID, C_IN * 9)                           # [64, 27]
    w1s = np.zeros((128, 64), f)
    for s in range(4):
        w1s[32 * s : 32 * s + 27] = w1.T
    w2 = conv2_w.reshape(C2, HID)                                 # [16, 64]
    w2b = np.zeros((128, 32), f)
    w2b[0:64, 0:16] = w2.T
    w2b[64:128, 16:32] = w2.T
    Wc3 = W_combo.reshape(ACT_DIM, C2, PIX)                       # [5, 16, 441]
    wcf = np.zeros((128, NR, ACT_DIM), f)
    for G in range(NR):
        for q in range(8):
            p = 8 * G + q
            if p < PIX:
                wcf[16 * q : 16 * (q + 1), G, :] = Wc3[:, :, p].T
    wc = wcf.reshape(128, NR * ACT_DIM)
    b1t = np.tile(conv1_b, 2).reshape(128, 1).astype(f)
    b2t = np.tile(conv2_b, 8).reshape(128, 1).astype(f)

    # im2col: K27[(c,dh,dw), pix, img]
    from numpy.lib.stride_tricks import sliding_window_view

    obs_im = obs.reshape(N * K, C_IN, H, W)
    obs_p = np.pad(obs_im, ((0, 0), (0, 0), (1, 1), (1, 1)))
    win = sliding_window_view(obs_p, (3, 3), axis=(2, 3))         # [NK, 3, 21, 21, 3, 3]
    K27 = win.transpose(1, 4, 5, 2, 3, 0).reshape(27, PIX, N * K)
    K27p = np.zeros((27, PIXP, N * K), f)
    K27p[:, :PIX] = K27
    Kv = K27p.reshape(27, NR, 2, 4, N * K)                        # (k, R, h, s, img)

    eye5 = np.eye(ACT_DIM, dtype=f)

    in_maps = []
    for r in range(N_CORES):
        i0, i1 = r * I_PC, (r + 1) * I_PC
        ximr = np.zeros((NR, 128, 2 * I_PC), BF16)
        for s in range(4):
            blk = Kv[:, :, :, s, i0:i1].transpose(1, 0, 2, 3).reshape(NR, 27, 2 * I_PC)
            ximr[:, 32 * s : 32 * s + 27, :] = blk.astype(BF16)
        a0 = r * A_PC
        mTr = np.ascontiguousarray(
            mask[a0 : a0 + A_PC, :].T.reshape(4, 128, A_PC)
        ).astype(f)
        in_maps.append(
            {
                "xim": ximr,
                "w1s": w1s.astype(BF16),
                "w2b": w2b.astype(BF16),
                "wc": wc.astype(BF16),
                "b1": b1t,
                "b2": b2t,
                "mT": mTr,
                "hb": np.ascontiguousarray(hbias[a0 : a0 + A_PC]).astype(f),
                "ey": eye5,
            }
        )
    return in_maps


def kernel(**inputs):
    global LAST_RESULT
    _ensure_ntff_hook()
    in_maps = _host_prep(**inputs)
    import time as _time
    print("build start", _time.strftime("%H:%M:%S"), flush=True)
    if "nc" not in _CACHE:
        _CACHE["nc"] = _build_nc()
    nc = _CACHE["nc"]
    print("build done", _time.strftime("%H:%M:%S"), flush=True)
    import time as _time
    print("run start", _time.strftime("%H:%M:%S"), flush=True)
    res = run_bass_kernel_spmd(nc, in_maps, core_ids=list(range(N_CORES)))
    print("run done", _time.strftime("%H:%M:%S"), flush=True)
    LAST_RESULT = res
    outp = np.concatenate([res.results[r]["out"] for r in range(N_CORES)], axis=0)
    return outp.astype(np.float32)


# revision 14
# speedup vs baseline: 1.4164x; 1.0321x over previous
import sys

sys.path.insert(0, "/opt/trn_rl_repo")
import numpy as np
import ml_dtypes

from concourse import bacc, tile, mybir
from concourse.bass_utils import run_bass_kernel_spmd

BF16 = ml_dtypes.bfloat16
N_CORES = 8
N, K, C_IN, H, W = 512, 4, 3, 21, 21
HID, C2, ACT_DIM = 64, 16, 5
OBS_R = (H // 2, W // 2)
PIX = H * W          # 441
PIXP = 448           # padded pixel count (multiple of 8)
NR = PIXP // 8       # 56 rounds of 8 pixels
A_PC = N // N_CORES  # 64 agents per core
I_PC = A_PC * K      # 256 images per core

_CACHE = {}
LAST_RESULT = None

import os as _os

F_WARM = _os.environ.get("K_WARM", "0") == "1"
F_PIPE = _os.environ.get("K_PIPE", "1") == "1"
F_DMAEND = _os.environ.get("K_DMAEND", "1") == "1"


def _ensure_ntff_hook():
    """This image's antenv lacks axon_hooks; inject a shim so
    run_bass_kernel_spmd's trace path works (exec_time_ns)."""
    import types

    try:
        import antenv

        if hasattr(antenv, "axon_hooks"):
            return
        from trn_agent_boot.trn_boot import _ntff_profile_via_ctypes

        mod = types.ModuleType("antenv.axon_hooks")
        _h = [_ntff_profile_via_ctypes("/opt/axon/libaxon_pjrt.so")]
        mod.set_axon_ntff_profile_hook = lambda h: _h.__setitem__(0, h)
        mod.get_axon_ntff_profile_hook = lambda: _h[0]
        sys.modules["antenv.axon_hooks"] = mod
        antenv.axon_hooks = mod
    except Exception:
        pass


def _build_nc():
    f32 = mybir.dt.float32
    bf16 = mybir.dt.bfloat16
    RELU = mybir.ActivationFunctionType.Relu
    ADD = mybir.AluOpType.add
    MAX = mybir.AluOpType.max

    nc = bacc.Bacc("TRN2", target_bir_lowering=False, debug=False, num_devices=N_CORES)
    xim = nc.declare_dram_parameter("xim", [NR, 128, 2 * I_PC], bf16, isOutput=False)
    w1s = nc.declare_dram_parameter("w1s", [128, 64], bf16, isOutput=False)
    w2b = nc.declare_dram_parameter("w2b", [128, 32], bf16, isOutput=False)
    wc = nc.declare_dram_parameter("wc", [128, NR * ACT_DIM], bf16, isOutput=False)
    b1 = nc.declare_dram_parameter("b1", [128, 1], f32, isOutput=False)
    b2 = nc.declare_dram_parameter("b2", [128, 1], f32, isOutput=False)
    mT = nc.declare_dram_parameter("mT", [4, 128, A_PC], f32, isOutput=False)
    hb = nc.declare_dram_parameter("hb", [A_PC, ACT_DIM], f32, isOutput=False)
    ey = nc.declare_dram_parameter("ey", [ACT_DIM, ACT_DIM], f32, isOutput=False)
    out = nc.declare_dram_parameter("out", [A_PC, ACT_DIM], f32, isOutput=True)

    with tile.TileContext(nc) as tc:
        with (
            tc.tile_pool(name="w", bufs=1) as wp,
            tc.tile_pool(name="x", bufs=2) as xp,
            tc.tile_pool(name="r", bufs=10) as rp,
            tc.tile_pool(name="pp", bufs=3) as pp,
            tc.tile_pool(name="sm", bufs=1) as sm,
            tc.tile_pool(name="ps1", bufs=4, space="PSUM") as ps1,
            tc.tile_pool(name="ps2", bufs=2, space="PSUM") as ps2,
            tc.tile_pool(name="psp", bufs=1, space="PSUM") as psp,
            tc.tile_pool(name="psq", bufs=1, space="PSUM") as psq,
            tc.tile_pool(name="dram", bufs=1, space="DRAM") as dp,
        ):
            w1t = wp.tile([128, 64], bf16)
            nc.sync.dma_start(w1t[:], w1s[:])
            w2t = wp.tile([128, 32], bf16)
            nc.sync.dma_start(w2t[:], w2b[:])
            wct = wp.tile([128, NR * ACT_DIM], bf16)
            nc.sync.dma_start(wct[:], wc[:])
            b1t = wp.tile([128, 1], f32)
            nc.sync.dma_start(b1t[:], b1[:])
            b2t = wp.tile([128, 1], f32)
            nc.sync.dma_start(b2t[:], b2[:])

            # projection accumulator, lives for the whole image loop
            qacc = psp.tile([ACT_DIM, I_PC], f32, tag="qacc")

            # PE warm-up: dummy matmuls while the first input DMA is in flight
            # (HAM unthrottles 1.2 -> 2.4 GHz after ~3.4us of sustained work)
            if F_WARM:
                warm = psq.tile([64, NR * ACT_DIM], f32, tag="q")
                for _ in range(24):
                    nc.tensor.matmul(
                        warm[:], w1t[:, :], wct[:, :], start=True, stop=True
                    )

            SG = 4  # rounds per DMA super-group (512 KiB per dma_start)
            D = 2 if F_PIPE else 0   # conv2 delay (software pipeline depth)
            prev = {}         # R -> (rA, rB)
            pend = None       # (pair, pt) projection deferred one pair
            psC = None
            for R in range(NR + D):
                if R < NR:
                    if R % SG == 0:
                        g = R // SG
                        xt = xp.tile([128, SG, 2 * I_PC], bf16, tag="xt")
                        if g == 0:
                            h = SG // 2
                            nc.sync.dma_start(
                                xt[:, 0:h, :],
                                xim[0:h, :, :].rearrange("r p c -> p r c"),
                            )
                            nc.sync.dma_start(
                                xt[:, h:SG, :],
                                xim[h:SG, :, :].rearrange("r p c -> p r c"),
                            )
                        else:
                            nc.sync.dma_start(
                                xt[:],
                                xim[g * SG : (g + 1) * SG, :, :].rearrange(
                                    "r p c -> p r c"
                                ),
                            )
                    rr = R % SG
                    # conv1: 4 row-strip matmuls; strips 0,1 -> bank A, 2,3 -> bank B
                    psA = ps1.tile([128, 2 * I_PC], f32, tag="c1")
                    psB = ps1.tile([128, 2 * I_PC], f32, tag="c1")
                    for s in range(4):
                        bank = psA if s < 2 else psB
                        e = s % 2
                        nc.tensor.matmul(
                            bank[64 * e : 64 * (e + 1), :],
                            w1t[32 * s : 32 * s + 27, :],
                            xt[32 * s : 32 * s + 27, rr, :],
                            start=True,
                            stop=True,
                            tile_position=(32 * s, 64 * e),
                        )
                    # relu1 + bias, psum -> sbuf bf16; split across ACT and DVE
                    rA = rp.tile([128, 2 * I_PC], bf16, tag="r")
                    rB = rp.tile([128, 2 * I_PC], bf16, tag="r")
                    nc.scalar.activation(rA[:], psA[:], RELU, bias=b1t[:, 0:1])
                    nc.vector.tensor_scalar(rB[:], psB[:], b1t[:, 0:1], 0.0, ADD, MAX)
                    prev[R] = (rA, rB)
                # conv2 for round R-D (pipelined: PE starts conv1(R) while
                # relu1(R-D) drains on ACT/DVE)
                Rp = R - D
                if 0 <= Rp < NR:
                    rAp, rBp = prev.pop(Rp)
                    if Rp % 2 == 0:
                        psC = ps2.tile([128, 2, I_PC], f32, tag="c2")
                    for j in range(4):
                        src = rAp if j % 2 == 0 else rBp
                        hh = j // 2
                        nc.tensor.matmul(
                            psC[32 * j : 32 * (j + 1), Rp % 2, :],
                            w2t[:, :],
                            src[:, hh * I_PC : (hh + 1) * I_PC],
                            start=True,
                            stop=True,
                            tile_position=(0, 32 * j),
                        )
                    if Rp % 2 == 1:
                        # relu2 + bias for two rounds at once; alternate engines
                        pt = pp.tile([128, 2, I_PC], bf16, tag="pt")
                        if (Rp // 2) % 2 == 0:
                            nc.scalar.activation(pt[:], psC[:], RELU, bias=b2t[:, 0:1])
                        else:
                            nc.vector.tensor_scalar(
                                pt[:], psC[:], b2t[:, 0:1], 0.0, ADD, MAX
                            )
                        if not F_PIPE:
                            P = Rp // 2
                            for t in range(2):
                                Rq = 2 * P + t
                                nc.tensor.matmul(
                                    qacc[:],
                                    wct[:, Rq * ACT_DIM : (Rq + 1) * ACT_DIM],
                                    pt[:, t, :],
                                    start=(Rq == 0),
                                    stop=(Rq == NR - 1),
                                )
                        else:
                            if pend is not None:
                                P, ptp = pend
                                for t in range(2):
                                    Rq = 2 * P + t
                                    nc.tensor.matmul(
                                        qacc[:],
                                        wct[:, Rq * ACT_DIM : (Rq + 1) * ACT_DIM],
                                        ptp[:, t, :],
                                        start=(Rq == 0),
                                        stop=False,
                                    )
                            pend = (Rp // 2, pt)
            if F_PIPE:
                # flush the last pending projection pair
                P, ptp = pend
                for t in range(2):
                    Rq = 2 * P + t
                    nc.tensor.matmul(
                        qacc[:],
                        wct[:, Rq * ACT_DIM : (Rq + 1) * ACT_DIM],
                        ptp[:, t, :],
                        start=False,
                        stop=(Rq == NR - 1),
                    )
            if not F_DMAEND:
                pass  # (placeholder; loads below happen here regardless)
            mt = wp.tile([128, 4, A_PC], f32)
            nc.sync.dma_start(mt[:], mT[:].rearrange("k j i -> j k i"))
            hbt = wp.tile([A_PC, ACT_DIM], f32)
            nc.sync.dma_start(hbt[:], hb[:])
            eyt = wp.tile([ACT_DIM, ACT_DIM], f32)
            nc.sync.dma_start(eyt[:], ey[:])
            # mean over K folded as sum (wc pre-scaled by 1/K): q5[5, 64]
            q5 = sm.tile([ACT_DIM, A_PC], f32, tag="q5")
            nc.vector.tensor_reduce(
                q5[:],
                qacc[:].rearrange("p (a k) -> p a k", k=K),
                mybir.AxisListType.X,
                ADD,
            )
            # transpose to [64, 5] via PE (q5.T @ I)
            psT = psq.tile([A_PC, ACT_DIM], f32, tag="q")
            nc.tensor.matmul(psT[:], q5[:], eyt[:], start=True, stop=True)
            qsb = sm.tile([A_PC, ACT_DIM], f32, tag="qsb")
            nc.vector.tensor_copy(qsb[:], psT[:])

            agi = dp.tile([A_PC, ACT_DIM], f32)
            ago = dp.tile([N, ACT_DIM], f32)
            nc.gpsimd.dma_start(agi[:], qsb[:])
            nc.gpsimd.collective_compute(
                "AllGather",
                mybir.AluOpType.bypass,
                replica_groups=[list(range(N_CORES))],
                ins=[agi[:].opt()],
                outs=[ago[:].opt()],
            )
            qall = sm.tile([128, 4, ACT_DIM], f32, tag="qall")
            nc.gpsimd.dma_start(qall[:], ago[:].rearrange("(k j) a -> j k a", j=128))

            # masked aggregation: Q[i, a] = sum_j mask[i, j] q[j, a]
            psM = psq.tile([A_PC, ACT_DIM], f32, tag="q")
            for kc in range(4):
                nc.tensor.matmul(
                    psM[:],
                    mt[:, kc, :],
                    qall[:, kc, :],
                    start=(kc == 0),
                    stop=(kc == 3),
                )
            osb = sm.tile([A_PC, ACT_DIM], f32, tag="osb")
            nc.vector.tensor_tensor(osb[:], psM[:], hbt[:], ADD)
            nc.sync.dma_start(out[:], osb[:])

    nc.compile()
    return nc


def _host_prep(obs, action, state, conv1_w, conv1_b, conv2_w, conv2_b,
               obs_w, obs_b, act_w, act_b, val_w, val_b, adv_w, adv_b):
    f = np.float32
    obs = np.asarray(obs, f)
    action = np.asarray(action).astype(np.int64)
    state = np.asarray(state).astype(np.int64)
    conv1_w = np.asarray(conv1_w, f)
    conv1_b = np.asarray(conv1_b, f)
    conv2_w = np.asarray(conv2_w, f)
    conv2_b = np.asarray(conv2_b, f)
    obs_w = np.asarray(obs_w, f)
    obs_b = np.asarray(obs_b, f)
    act_w = np.asarray(act_w, f)
    act_b = np.asarray(act_b, f)
    val_w = np.asarray(val_w, f)
    val_b = np.asarray(val_b, f)
    adv_w = np.asarray(adv_w, f)
    adv_b = np.asarray(adv_b, f)

    # dueling head folded into a single linear: Q = latent @ Wq.T + bq
    Wq = val_w[0][None, :] + adv_w - adv_w.mean(axis=0)[None, :]  # [5, 32]
    bq = val_b[0] + adv_b - adv_b.mean()                          # [5]
    Wqo, Wqa = Wq[:, :16], Wq[:, 16:]
    W_combo = (Wqo @ obs_w) / K                                   # [5, 7056]

    aoh = np.zeros((N, ACT_DIM), f)
    aoh[np.arange(N), action] = 1.0
    a_enc = aoh @ act_w.T + act_b                                 # [512, 16]
    h = obs_b @ Wqo.T + a_enc @ Wqa.T                             # [512, 5]

    d = np.abs(state[:, None, :] - state[None, :, :])
    within = (d[..., 0] <= OBS_R[0]) & (d[..., 1] <= OBS_R[1])
    upper = np.triu(np.ones((N, N), bool), 1)
    mask = (np.eye(N, dtype=bool) | (within & upper)).astype(f)   # [512, 512]
    hbias = mask @ h + bq[None, :]                                # [512, 5]

    # device weight layouts
    w1 = conv1_w.reshape(HID, C_IN * 9)                           # [64, 27]
    w1s = np.zeros((128, 64), f)
    for s in range(4):
        w1s[32 * s : 32 * s + 27] = w1.T
    w2 = conv2_w.reshape(C2, HID)                                 # [16, 64]
    w2b = np.zeros((128, 32), f)
    w2b[0:64, 0:16] = w2.T
    w2b[64:128, 16:32] = w2.T
    Wc3 = W_combo.reshape(ACT_DIM, C2, PIX)                       # [5, 16, 441]
    wcf = np.zeros((128, NR, ACT_DIM), f)
    for G in range(NR):
        for q in range(8):
            p = 8 * G + q
            if p < PIX:
                wcf[16 * q : 16 * (q + 1), G, :] = Wc3[:, :, p].T
    wc = wcf.reshape(128, NR * ACT_DIM)
    b1t = np.tile(conv1_b, 2).reshape(128, 1).astype(f)
    b2t = np.tile(conv2_b, 8).reshape(128, 1).astype(f)

    # im2col: K27[(c,dh,dw), pix, img]
    from numpy.lib.stride_tricks import sliding_window_view

    obs_im = obs.reshape(N * K, C_IN, H, W)
    obs_p = np.pad(obs_im, ((0, 0), (0, 0), (1, 1), (1, 1)))
    win = sliding_window_view(obs_p, (3, 3), axis=(2, 3))         # [NK, 3, 21, 21, 3, 3]
    K27 = win.transpose(1, 4, 5, 2, 3, 0).reshape(27, PIX, N * K)
    K27p = np.zeros((27, PIXP, N * K), f)
    K27p[:, :PIX] = K27
    Kv = K27p.reshape(27, NR, 2, 4, N * K)                        # (k, R, h, s, img)

    eye5 = np.eye(ACT_DIM, dtype=f)

    in_maps = []
    for r in range(N_CORES):
        i0, i1 = r * I_PC, (r + 1) * I_PC
        ximr = np.zeros((NR, 128, 2 * I_PC), BF16)
        for s in range(4):
            blk = Kv[:, :, :, s, i0:i1].transpose(1, 0, 2, 3).reshape(NR, 27, 2 * I_PC)
            ximr[:, 32 * s : 32 * s + 27, :] = blk.astype(BF16)
        a0 = r * A_PC
        mTr = np.ascontiguousarray(
            mask[a0 : a0 + A_PC, :].T.reshape(4, 128, A_PC)
        ).astype(f)
        in_maps.append(
            {
                "xim": ximr,
                "w1s": w1s.astype(BF16),
                "w2b": w2b.astype(BF16),
                "wc": wc.astype(BF16),
                "b1": b1t,
                "b2": b2t,
                "mT": mTr,
                "hb": np.ascontiguousarray(hbias[a0 : a0 + A_PC]).astype(f),
                "ey": eye5,
            }
        )
    return in_maps


def kernel(**inputs):
    global LAST_RESULT
    _ensure_ntff_hook()
    in_maps = _host_prep(**inputs)
    import time as _time
    print("build start", _time.strftime("%H:%M:%S"), flush=True)
    if "nc" not in _CACHE:
        _CACHE["nc"] = _build_nc()
    nc = _CACHE["nc"]
    print("build done", _time.strftime("%H:%M:%S"), flush=True)
    import time as _time
    print("run start", _time.strftime("%H:%M:%S"), flush=True)
    res = run_bass_kernel_spmd(nc, in_maps, core_ids=list(range(N_CORES)))
    print("run done", _time.strftime("%H:%M:%S"), flush=True)
    LAST_RESULT = res
    outp = np.concatenate([res.results[r]["out"] for r in range(N_CORES)], axis=0)
    return outp.astype(np.float32)
